# revision 10
# baseline (speedup 1.0000x reference)
"""Trainium2 Bass kernel for the CRule loss.

Math (identical to the reference, restructured):
    Hs = H @ y_pred.T                       # [C, B]
    loss[b] = (sum_c Hs[c,b] - y_pred[b,:] @ Hs[:,b]) / sum(H)
            = (y_pred[b,:] . colsum(H) - y_pred[b,:] @ H @ y_pred[b,:]^T) / sum(H)

Rewriting with  Z = y_pred @ H  and  colsum(H)[k] = sum_c H[c,k]:
    loss[b] = -(1/sumH) * sum_k y[b,k] * (Z[b,k] - colsum[k])

The (Z - colsum_bcast) term is accumulated directly in PSUM: each tile's
matmul group is one rank-1 update (ones ⊗ (-colsum), start=True) plus eight
contraction-chunk matmuls.  A single scalar_tensor_tensor per output half
then computes sum_k y*(Z-colsum) fused, and the scalar engine applies
-1/sumH.

y_true is unused by the reference and therefore ignored here.

Sharding: data-parallel over the batch dim across 8 cores (2048 rows each),
H replicated in every core's SBUF. No collectives.

Matmuls run as float32r (reduced-precision fp32 multiply, full PE rate for
moving dims >= 256, ~5e-5 relative error).
"""

import os

import numpy as np

import concourse.bass as bass
import concourse.mybir as mybir
from concourse import bacc
from concourse.bass_utils import run_bass_kernel_spmd
from concourse.masks import make_identity
from concourse.tile import TileContext

B = 16384
C = 1000
N_CORES = 8
B_SH = B // N_CORES  # 2048 rows per core
P = 128
NB = B_SH // P       # 16 batch tiles per core
CK = 8               # contraction chunks
CKS = C // CK        # 125 (partition dim of transposed chunks)
KN = 2               # output-column halves
KNS = C // KN        # 500 (fits one PSUM bank in fp32)

F32 = mybir.dt.float32
F32R = mybir.dt.float32r
MULT = mybir.AluOpType.mult
ADD = mybir.AluOpType.add

_CACHE = {}
LAST_RESULTS = None


def _build():
    nc = bacc.Bacc()
    y = nc.dram_tensor("y", [B_SH, C], F32, kind="ExternalInput")
    h = nc.dram_tensor("h", [C, C], F32R, kind="ExternalInput")
    out = nc.dram_tensor("loss_out", [P, NB], F32, kind="ExternalOutput")

    with TileContext(nc) as tc:
        with (
            tc.tile_pool(name="const", bufs=1) as constp,
            tc.tile_pool(name="hpool", bufs=1) as hp,
            tc.tile_pool(name="ypool", bufs=3) as yp,
            tc.tile_pool(name="ytpool", bufs=2) as ytp,
            tc.tile_pool(name="scr", bufs=2) as scrp,
            tc.tile_pool(name="accs", bufs=4) as accp,
            tc.tile_pool(name="pt", bufs=3, space="PSUM") as ptp,
            tc.tile_pool(name="pz", bufs=4, space="PSUM") as pzp,
        ):
            ident = constp.tile([P, P], F32)
            make_identity(nc, ident)
            ones_cf = constp.tile([P, 1], F32)
            nc.gpsimd.memset(ones_cf, 1.0)
            ones_rf = constp.tile([1, P], F32)
            nc.gpsimd.memset(ones_rf, 1.0)
            ones_col = constp.tile([P, 1], F32R)
            nc.vector.tensor_copy(ones_col, ones_cf)
            ones_row = constp.tile([1, P], F32R)
            nc.vector.tensor_copy(ones_row, ones_rf)

            # H chunks: h_sb[:, ck*C:(ck+1)*C] = H[ck*125:(ck+1)*125, :]
            # single DMA (3D AP) so consumers wait on one semaphore
            h_sb = hp.tile([CKS, CK * C], F32R)
            h_re = h[:, :].rearrange("(ck p) k -> p ck k", p=CKS)
            nc.sync.dma_start(
                out=h_sb.rearrange("p (ck k) -> p ck k", ck=CK),
                in_=h_re,
            )

            # colsum(H)[k] = sum_c H[c,k]  -> [1, C]
            cs_f = constp.tile([1, C], F32)
            for kn in range(KN):
                cs_ps = pzp.tile([1, KNS], F32, tag="z", name=f"cs_ps{kn}")
                for ck in range(CK):
                    nc.tensor.matmul(
                        cs_ps,
                        lhsT=ones_col[0:CKS, :],
                        rhs=h_sb[:, ck * C + kn * KNS: ck * C + (kn + 1) * KNS],
                        start=(ck == 0),
                        stop=(ck == CK - 1),
                    )
                nc.vector.tensor_copy(cs_f[:, kn * KNS:(kn + 1) * KNS], cs_ps)

            # -colsum as f32r (rank-1 accumulation operand)
            cs_neg = constp.tile([1, C], F32R)
            nc.vector.tensor_scalar_mul(cs_neg, cs_f, -1.0)

            # -1/sumH broadcast across partitions
            sum_h = constp.tile([1, 1], F32)
            nc.vector.reduce_sum(sum_h, cs_f, axis=mybir.AxisListType.X)
            recip_f = constp.tile([1, 1], F32)
            nc.vector.reciprocal(recip_f, sum_h)
            nrecip = constp.tile([1, 1], F32)
            nc.vector.tensor_scalar_mul(nrecip, recip_f, -1.0)
            recip_ps = ptp.tile([P, 1], F32, tag="t", name="recip_ps")
            nc.tensor.matmul(recip_ps, lhsT=ones_rf, rhs=nrecip, start=True, stop=True)
            nrecip_bc = constp.tile([P, 1], F32)
            nc.vector.tensor_copy(nrecip_bc, recip_ps)

            loss_acc = constp.tile([P, NB], F32)

            for i in range(NB):
                y_tile = yp.tile([P, C], F32, name="y_tile")
                nc.sync.dma_start(out=y_tile, in_=y[i * P:(i + 1) * P, :])

                # transpose the 8 [128,125] chunks -> ytT [125, 8*128]
                yt = ytp.tile([CKS, CK * P], F32R, name="yt")
                for ck in range(CK):
                    pt = ptp.tile([CKS, P], F32, tag="t", name="pt")
                    nc.tensor.transpose(
                        pt,
                        y_tile[:, ck * CKS:(ck + 1) * CKS],
                        ident,
                    )
                    nc.any.tensor_copy(yt[:, ck * P:(ck + 1) * P], pt)

                # PSUM <- ones ⊗ (-colsum) + sum_ck ytT_ck^T @ H_ck = Z - colsum
                s_half = []
                for kn in range(KN):
                    pz = pzp.tile([P, KNS], F32, tag="z", name="pz")
                    nc.tensor.matmul(
                        pz,
                        lhsT=ones_row,
                        rhs=cs_neg[:, kn * KNS:(kn + 1) * KNS],
                        start=True,
                        stop=False,
                    )
                    for ck in range(CK):
                        nc.tensor.matmul(
                            pz,
                            lhsT=yt[:, ck * P:(ck + 1) * P],
                            rhs=h_sb[:, ck * C + kn * KNS: ck * C + (kn + 1) * KNS],
                            start=False,
                            stop=(ck == CK - 1),
                        )
                    # s_kn = sum_k y * (Z - colsum)   (fused multiply-reduce)
                    s_o = accp.tile([P, 1], F32, name="s_o")
                    scr = scrp.tile([P, KNS], F32, name="scr")
                    nc.vector.scalar_tensor_tensor(
                        out=scr,
                        in0=pz,
                        scalar=1.0,
                        in1=y_tile[:, kn * KNS:(kn + 1) * KNS],
                        op0=MULT,
                        op1=MULT,
                        accum_out=s_o,
                    )
                    s_half.append(s_o)

                tot = accp.tile([P, 1], F32, name="tot")
                nc.vector.tensor_add(tot, s_half[0], s_half[1])
                # loss = -(1/sumH) * tot
                nc.scalar.mul(loss_acc[:, i:i + 1], tot, nrecip_bc)

            nc.sync.dma_start(out=out[:, :], in_=loss_acc)

    if not nc.is_finalized():
        nc.finalize()
    return nc


def kernel(**inputs):
    global LAST_RESULTS
    y_pred = np.ascontiguousarray(np.asarray(inputs["y_pred"]), dtype=np.float32)
    H = np.ascontiguousarray(np.asarray(inputs["H"]), dtype=np.float32)
    assert y_pred.shape == (B, C) and H.shape == (C, C)

    nc = _CACHE.get("nc")
    if nc is None:
        nc = _build()
        _CACHE["nc"] = nc

    in_maps = [
        {"y": np.ascontiguousarray(y_pred[s * B_SH:(s + 1) * B_SH]), "h": H}
        for s in range(N_CORES)
    ]
    res = run_bass_kernel_spmd(
        nc,
        in_maps,
        core_ids=list(range(N_CORES)),
        trace=bool(int(os.environ.get("KBENCH_TRACE", "0"))),
    )
    LAST_RESULTS = res
    # loss_out is [128, 16] partition-major: element [p, i] = loss for shard
    # row i*128 + p. Transpose+flatten restores batch order per shard.
    loss = np.concatenate(
        [np.asarray(r["loss_out"]).T.reshape(-1) for r in res.results]
    ).astype(np.float32)
    return loss


# revision 11
# speedup vs baseline: 1.2226x; 1.2226x over previous
"""Trainium2 Bass kernel for the CRule loss.

Math (identical to the reference, restructured):
    Hs = H @ y_pred.T                       # [C, B]
    loss[b] = (sum_c Hs[c,b] - y_pred[b,:] @ Hs[:,b]) / sum(H)
            = (y_pred[b,:] . colsum(H) - y_pred[b,:] @ H @ y_pred[b,:]^T) / sum(H)

Rewriting with  Z = y_pred @ H  and  colsum(H)[k] = sum_c H[c,k]:
    loss[b] = -(1/sumH) * sum_k y[b,k] * (Z[b,k] - colsum[k])

Per 128-row batch tile, the (Z - colsum_bcast) term is accumulated directly
in PSUM: one rank-1 update (ones x (-colsum), start=True) plus eight K=128
matmuls whose stationary operand is the DMA-transposed y tile.  A single
scalar_tensor_tensor per 500-column half computes sum_k y*(Z-colsum) fused
on the vector engine, and the scalar engine applies -1/sumH.

y_true is unused by the reference and therefore ignored.

Sharding: data-parallel over the batch dim across 8 cores (2048 rows each),
H replicated in every core's SBUF. No collectives.

Precision: y and H are fed as bf16 (host-cast); matmuls accumulate fp32 in
PSUM; the colsum rank-1 correction runs as float32r.  End-to-end relative
error vs the fp32 reference is ~1e-4.

Layout trick: y is host-padded to 1024 columns so one hardware DMA-transpose
per tile (3D out AP [128, 8, 128]) yields all eight transposed contraction
chunks; H is host-padded to 1024 rows to match.
"""

import os

import numpy as np
import ml_dtypes

import concourse.bass as bass
import concourse.mybir as mybir
from concourse import bacc
from concourse.bass_utils import run_bass_kernel_spmd
from concourse.tile import TileContext

B = 16384
C = 1000
CP = 1024            # padded contraction dim
N_CORES = 8
B_SH = B // N_CORES  # 2048 rows per core
P = 128
NB = B_SH // P       # 16 batch tiles per core
CK = CP // P         # 8 contraction chunks of 128
KN = 2               # output-column halves
KNS = C // KN        # 500 (fits one PSUM bank in fp32)

F32 = mybir.dt.float32
F32R = mybir.dt.float32r
BF16 = mybir.dt.bfloat16
MULT = mybir.AluOpType.mult
ADD = mybir.AluOpType.add

_CACHE = {}
LAST_RESULTS = None


def _build():
    nc = bacc.Bacc()
    y = nc.dram_tensor("y", [B_SH, CP], BF16, kind="ExternalInput")
    h = nc.dram_tensor("h", [CP, C], BF16, kind="ExternalInput")
    out = nc.dram_tensor("loss_out", [P, NB], F32, kind="ExternalOutput")

    with TileContext(nc) as tc:
        with (
            tc.tile_pool(name="const", bufs=1) as constp,
            tc.tile_pool(name="hpool", bufs=1) as hp,
            tc.tile_pool(name="ypool", bufs=3) as yp,
            tc.tile_pool(name="ytpool", bufs=3) as ytp,
            tc.tile_pool(name="scr", bufs=2) as scrp,
            tc.tile_pool(name="accs", bufs=4) as accp,
            tc.tile_pool(name="pr", bufs=1, space="PSUM") as prp,
            tc.tile_pool(name="pz", bufs=4, space="PSUM") as pzp,
        ):
            ones_cf = constp.tile([P, 1], F32)
            nc.gpsimd.memset(ones_cf, 1.0)
            ones_rf = constp.tile([1, P], F32)
            nc.gpsimd.memset(ones_rf, 1.0)
            ones_col = constp.tile([P, 1], BF16)
            nc.vector.tensor_copy(ones_col, ones_cf)
            ones_row = constp.tile([1, P], F32R)
            nc.vector.tensor_copy(ones_row, ones_rf)

            # H chunks: h_sb[:, ck*C:(ck+1)*C] = H[ck*128:(ck+1)*128, :]
            # single DMA (3D AP) so consumers wait on one semaphore
            h_sb = hp.tile([P, CK * C], BF16)
            h_re = h[:, :].rearrange("(ck p) k -> p ck k", p=P)
            nc.sync.dma_start(
                out=h_sb.rearrange("p (ck k) -> p ck k", ck=CK),
                in_=h_re,
            )

            # colsum(H)[k] = sum_c H[c,k]  -> [1, C]
            cs_f = constp.tile([1, C], F32)
            for kn in range(KN):
                cs_ps = pzp.tile([1, KNS], F32, tag="z", name=f"cs_ps{kn}")
                for ck in range(CK):
                    nc.tensor.matmul(
                        cs_ps,
                        lhsT=ones_col,
                        rhs=h_sb[:, ck * C + kn * KNS: ck * C + (kn + 1) * KNS],
                        start=(ck == 0),
                        stop=(ck == CK - 1),
                    )
                nc.vector.tensor_copy(cs_f[:, kn * KNS:(kn + 1) * KNS], cs_ps)

            # -colsum as f32r (rank-1 accumulation operand)
            cs_neg = constp.tile([1, C], F32R)
            nc.vector.tensor_scalar_mul(cs_neg, cs_f, -1.0)

            # -1/sumH broadcast across partitions
            sum_h = constp.tile([1, 1], F32)
            nc.vector.reduce_sum(sum_h, cs_f, axis=mybir.AxisListType.X)
            recip_f = constp.tile([1, 1], F32)
            nc.vector.reciprocal(recip_f, sum_h)
            nrecip = constp.tile([1, 1], F32)
            nc.vector.tensor_scalar_mul(nrecip, recip_f, -1.0)
            recip_ps = prp.tile([P, 1], F32, name="recip_ps")
            nc.tensor.matmul(recip_ps, lhsT=ones_rf, rhs=nrecip, start=True, stop=True)
            nrecip_bc = constp.tile([P, 1], F32)
            nc.vector.tensor_copy(nrecip_bc, recip_ps)

            loss_acc = constp.tile([P, NB], F32)

            for i in range(NB):
                y_tile = yp.tile([P, CP], BF16, name="y_tile")
                nc.sync.dma_start(out=y_tile, in_=y[i * P:(i + 1) * P, :])

                # hardware transpose: yt[p, ck, b] = y[i*128+b, ck*128+p]
                yt = ytp.tile([P, CK * P], BF16, name="yt")
                nc.scalar.dma_start_transpose(
                    out=yt.rearrange("p (ck b) -> p ck b", ck=CK),
                    in_=y[i * P:(i + 1) * P, :],
                )

                # PSUM <- ones x (-colsum) + sum_ck ytT_ck^T @ H_ck = Z - colsum
                s_half = []
                for kn in range(KN):
                    pz = pzp.tile([P, KNS], F32, tag="z", name="pz")
                    nc.tensor.matmul(
                        pz,
                        lhsT=ones_row,
                        rhs=cs_neg[:, kn * KNS:(kn + 1) * KNS],
                        start=True,
                        stop=False,
                    )
                    for ck in range(CK):
                        nc.tensor.matmul(
                            pz,
                            lhsT=yt[:, ck * P:(ck + 1) * P],
                            rhs=h_sb[:, ck * C + kn * KNS: ck * C + (kn + 1) * KNS],
                            start=False,
                            stop=(ck == CK - 1),
                        )
                    # s_kn = sum_k y * (Z - colsum)   (fused multiply-reduce)
                    s_o = accp.tile([P, 1], F32, name="s_o")
                    scr = scrp.tile([P, KNS], F32, name="scr")
                    nc.vector.scalar_tensor_tensor(
                        out=scr,
                        in0=pz,
                        scalar=1.0,
                        in1=y_tile[:, kn * KNS:(kn + 1) * KNS],
                        op0=MULT,
                        op1=MULT,
                        accum_out=s_o,
                    )
                    s_half.append(s_o)

                tot = accp.tile([P, 1], F32, name="tot")
                nc.vector.tensor_add(tot, s_half[0], s_half[1])
                # loss = -(1/sumH) * tot
                nc.scalar.mul(loss_acc[:, i:i + 1], tot, nrecip_bc)

            nc.sync.dma_start(out=out[:, :], in_=loss_acc)

    if not nc.is_finalized():
        nc.finalize()
    return nc


def kernel(**inputs):
    global LAST_RESULTS
    y_pred = np.asarray(inputs["y_pred"])
    H = np.asarray(inputs["H"])
    assert y_pred.shape == (B, C) and H.shape == (C, C)

    # host-side layout/dtype prep: bf16 cast + contraction-dim zero-pad
    y_b = np.zeros((B, CP), dtype=ml_dtypes.bfloat16)
    y_b[:, :C] = y_pred.astype(ml_dtypes.bfloat16)
    h_b = np.zeros((CP, C), dtype=ml_dtypes.bfloat16)
    h_b[:C, :] = H.astype(ml_dtypes.bfloat16)

    nc = _CACHE.get("nc")
    if nc is None:
        nc = _build()
        _CACHE["nc"] = nc

    in_maps = [
        {"y": np.ascontiguousarray(y_b[s * B_SH:(s + 1) * B_SH]), "h": h_b}
        for s in range(N_CORES)
    ]
    res = run_bass_kernel_spmd(
        nc,
        in_maps,
        core_ids=list(range(N_CORES)),
        trace=bool(int(os.environ.get("KBENCH_TRACE", "0"))),
    )
    LAST_RESULTS = res
    # loss_out is [128, 16] partition-major: element [p, i] = loss for shard
    # row i*128 + p. Transpose+flatten restores batch order per shard.
    loss = np.concatenate(
        [np.asarray(r["loss_out"]).T.reshape(-1) for r in res.results]
    ).astype(np.float32)
    return loss
